# revision 18
# baseline (speedup 1.0000x reference)
"""MoE layer (top-k routing) on 8 Trainium2 NeuronCores.

Expert-parallel per the sharding hint: the host computes router softmax +
top-k (0.1% of FLOPs) and realizes the "all-to-all dispatch by expert
assignment" while building the per-core SPMD input maps; each core runs
expert FFN work (fp32 PSUM accumulation); the host applies the combine
weights and scatter-adds results back to [B,N,C].

Load balance: each expert's FFN is split along D_FF into four quarter-units
(exact: gelu is elementwise over F and GEMM2 contracts F, so the four
partial y's just add). The 32 quarter-units are assigned four per core, one
per slot class A-D: each slot holds one expert pair's quarters (cores 0-3
take the larger expert of the pair, 4-7 the smaller), padded to the pair
max, within ~1% of the perfect-balance floor.

Mixed precision: each expert's tokens are sorted by combine weight
ascending; the first n8 columns of each slot run both GEMMs as naked
fp8-e4m3 DoubleRow matmuls (virtual K=256, measured ~1.9x bf16 per tile),
the rest in bf16. A pair's output error is scaled by its combine weight, so
putting low-weight pairs on the fp8 path keeps the end-to-end rel err
~1.6-1.8e-2 (budget 2e-2) while cutting PE busy time ~11%.

All dram tensors use partition-contiguous tiled layouts ([P, tiles*CB, TN]
for x/y, [P, CB, FQ]-style for weights) so every DMA moves 4-16KB runs per
partition; the f-major rearranged layouts previously produced 0.25-1KB
packets that halved ring throughput and starved the PE at startup.

DMA rings: sync carries the first unit's fp8 weights (it is the earliest
queue to start, ~9us) then x tiles in consumption order and y stores;
scalar carries biases + the first unit's bf16 weights (no WAR waits ahead
of the activations); gpsimd carries units B-D weights, WAR-paced by the
double-buffered weight pools, which blocks nothing since gpsimd has no
other work. Units are computed largest-tail first so the final tile's
store drain is minimal.
"""

import json
import os
import sys
import types

import numpy as np
import ml_dtypes

D_MODEL = 1024
D_FF = 4096
N_EXPERTS = 8
N_CORES = 8

P = 128
CB = D_MODEL // P      # 8 c-blocks of 128
FQ = D_FF // 4         # F quarter = 1024
FBQ = FQ // P          # 8 f-blocks per quarter
TN = 512               # token tile (matmul moving free dim / one PSUM bank)
HT = 256               # half-tile granularity for the fp8 column budget
SLOTS = ("A", "B", "C", "D")
SW = 4096.0            # fp8 weight pre-scale (max |w|*SW ~ 140 < 240)
F8_COLS = 3584         # total fp8 token columns across the 4 slots (7 tiles)


def _shim_axon_hooks():
    """Register the NTFF profile hook bass_utils looks for under axon; the
    image's `antenv` stub lacks `axon_hooks`."""
    if "antenv.axon_hooks" in sys.modules:
        return
    try:
        import trn_agent_boot.trn_boot as _tb
        hook = _tb._ntff_profile_via_ctypes("/opt/axon/libaxon_pjrt.so")
    except Exception:
        hook = None
    mod = types.ModuleType("antenv.axon_hooks")
    mod.get_axon_ntff_profile_hook = lambda: hook
    mod.set_axon_ntff_profile_hook = lambda h: None
    sys.modules["antenv.axon_hooks"] = mod


_shim_axon_hooks()

import concourse.bass as bass            # noqa: E402
import concourse.tile as tile            # noqa: E402
from concourse import mybir              # noqa: E402
from concourse.bass import ds, ts        # noqa: E402
from concourse.bass_utils import run_bass_kernel_spmd  # noqa: E402


def _fix_multiwait_bir(nc):
    """Split instructions carrying >1 sync wait (the TileContext tail drain)
    into single-wait NoOps; this walrus build rejects multi-wait CTRL
    instructions."""
    raw = bass.Bass.to_json_bytes(nc)
    d = json.loads(raw)
    for f in d["functions"]:
        for b in f["blocks"]:
            out = []
            for i in b["instructions"]:
                si = i.get("sync_info") or {}
                waits = si.get("on_wait") or []
                if len(waits) > 1:
                    for k, w in enumerate(waits[:-1]):
                        out.append({
                            "name": f"{i['name']}_wsplit{k}",
                            "engine": i["engine"],
                            "ins": [], "outs": [],
                            "opcode": "NoOp",
                            "sync_info": {"on_update": [], "on_wait": [w]},
                        })
                    si["on_wait"] = [waits[-1]]
                out.append(i)
            b["instructions"] = out
    fixed = json.dumps(d).encode()
    nc.to_json_bytes = lambda: fixed


_NC_CACHE = {}


def _tiles_of(cap, n8):
    """(off, tw, is_fp8) tiles: fp8 columns first (512s then a possible 256
    half-tile), then bf16 512-tiles with a ragged tail."""
    tiles = []
    off = 0
    while off < n8:
        tw = min(TN, n8 - off)
        tiles.append((off, tw, True))
        off += tw
    while off < cap:
        tw = min(TN, cap - off)
        tiles.append((off, tw, False))
        off += tw
    return tiles


def _build_moe_kernel(cfg):
    """cfg: tuple of (cap, n8) per slot in compute order. SPMD x8."""
    if cfg in _NC_CACHE:
        return _NC_CACHE[cfg]

    bf16 = mybir.dt.float16
    f8 = mybir.dt.float8e4
    f32 = mybir.dt.float32
    Act = mybir.ActivationFunctionType
    DR = mybir.MatmulPerfMode.DoubleRow

    nc = bass.Bass("TRN2", target_bir_lowering=False, debug=False,
                   num_devices=N_CORES)

    units = []
    for slot, (cap, n8) in zip(SLOTS, cfg):
        u = {"cap": cap, "n8": n8, "slot": slot}
        u["tiles"] = _tiles_of(cap, n8)
        nt8 = sum(1 for t in u["tiles"] if t[2])
        ntb = len(u["tiles"]) - nt8
        u["nt8"], u["ntb"] = nt8, ntb
        if n8:
            u["xT8"] = nc.declare_dram_parameter(f"xT8{slot}", [P, nt8 * CB, TN], f8, isOutput=False)
            u["w1q8"] = nc.declare_dram_parameter(f"w1q8{slot}", [P, CB, FQ], f8, isOutput=False)
            u["w2q8"] = nc.declare_dram_parameter(f"w2q8{slot}", [P, FBQ, D_MODEL], f8, isOutput=False)
        if ntb:
            u["xT"] = nc.declare_dram_parameter(f"xT{slot}", [P, ntb * CB, TN], bf16, isOutput=False)
        u["w1t"] = nc.declare_dram_parameter(f"w1t{slot}", [P, CB, FQ], bf16, isOutput=False)
        u["w2t"] = nc.declare_dram_parameter(f"w2t{slot}", [P, FBQ, D_MODEL], bf16, isOutput=False)
        # partials return as bf16: halves the output DMA so total traffic
        # stays under the chip's P0 power-throttle trigger; host sums in f32
        u["yT"] = nc.declare_dram_parameter(f"yT{slot}", [P, len(u["tiles"]) * CB, TN], bf16, isOutput=True)
        units.append(u)
    # all biases in one partition-contiguous tensor: the old per-slot
    # "(g p) -> p g" rearrange emitted 8192 four-byte DMA packets that
    # clogged a ring for tens of us
    bias_d = nc.declare_dram_parameter("biases", [P, 4 * 2 * FBQ], f32, isOutput=False)

    ua = units[0]
    with tile.TileContext(nc) as tc:
        with (
            tc.tile_pool(name="wbf", bufs=2) as wbf,
            tc.tile_pool(name="wq8", bufs=2) as wq8,
            tc.tile_pool(name="bias", bufs=1) as bpool,
            tc.tile_pool(name="xin", bufs=4) as xpool,
            tc.tile_pool(name="x8in", bufs=4) as x8pool,
            tc.tile_pool(name="hbuf", bufs=2) as hpool,
            tc.tile_pool(name="h8buf", bufs=2) as h8pool,
            tc.tile_pool(name="yout", bufs=2) as ypool,
            tc.tile_pool(name="psum", bufs=4, space="PSUM") as psum,
        ):
            # ---- weight/bias loads.
            # unit A's startup-critical fp8 tensors are split by partition
            # halves across the sync (starts ~9us) and scalar (~11us) rings:
            # the per-queue packet rate caps one ring at ~130-190GB/s, so
            # two queues halve the time to the first matmul. biases go
            # first on gpsimd (one 32KB dma). all bf16/deferred weights go
            # on gpsimd, WAR-paced by the double-buffered pools, which
            # blocks nothing since gpsimd has no other work.
            bt = bpool.tile([P, 4 * 2 * FBQ], f32, tag="bias", name="bias")
            nc.gpsimd.dma_start(bt[:, :], bias_d.ap())
            for ui, u in enumerate(units):
                u["b1ap"] = lambda m, _s=ui * 2 * FBQ: bt[:, ds(_s + m, 1)]
                u["b2ap"] = lambda c, _s=ui * 2 * FBQ + FBQ: bt[:, ds(_s + c, 1)]
            u = None
            # flat tile list in natural order (each unit: fp8 tiles then
            # bf16); x loads are issued 3 tiles ahead of consumption so no
            # x trigger queues behind a y-store trigger whose data is not
            # ready yet (the in-order sync engine would collapse the
            # prefetch pipeline at unit boundaries and at the drain)
            flat = []
            for ui_, u_ in enumerate(units):
                for ti_, tt in enumerate(u_["tiles"]):
                    flat.append((ui_, ti_) + tt)
            pre = {}
            # startup-critical unit-A tensors split by partition halves
            # across the sync (starts ~9us) and scalar (~11us) rings: the
            # per-queue packet rate caps one ring at ~130-190GB/s
            H = P // 2
            if ua["n8"]:
                ua["w1q8_sb"] = wq8.tile([P, CB, FQ], f8, tag="w1q8", name="w1q8A")
                ua["w2q8_sb"] = wq8.tile([P, FBQ, D_MODEL], f8, tag="w2q8", name="w2q8A")
                x0 = x8pool.tile([P, CB, TN], f8, tag="x8", name="x0")
                nc.sync.dma_start(ua["w1q8_sb"][0:H, :, :], ua["w1q8"].ap()[0:H, :, :])
                nc.scalar.dma_start(ua["w1q8_sb"][H:P, :, :], ua["w1q8"].ap()[H:P, :, :])
                nc.sync.dma_start(x0[0:H, :, :], ua["xT8"].ap()[0:H, ds(0, CB), :])
                nc.scalar.dma_start(x0[H:P, :, :], ua["xT8"].ap()[H:P, ds(0, CB), :])
                nc.sync.dma_start(ua["w2q8_sb"][0:H, :, :], ua["w2q8"].ap()[0:H, :, :])
                nc.scalar.dma_start(ua["w2q8_sb"][H:P, :, :], ua["w2q8"].ap()[H:P, :, :])
                pre[(0, 0)] = x0

            def _issue_x(fi, eng8=None):
                ui_, ti_, off_, tw_, is8_ = flat[fi]
                if (ui_, ti_) in pre:
                    return
                u_ = units[ui_]
                if is8_:
                    xt_ = x8pool.tile([P, CB, TN], f8, tag="x8")
                    (eng8 or nc.scalar).dma_start(xt_[:, :, :], u_["xT8"].ap()[:, ds(ti_ * CB, CB), :])
                else:
                    tb_ = ti_ - u_["nt8"]
                    xt_ = xpool.tile([P, CB, TN], bf16, tag="xt")
                    nc.sync.dma_start(xt_[:, :, :], u_["xT"].ap()[:, ds(tb_ * CB, CB), :])
                pre[(ui_, ti_)] = xt_

            for fi in range(min(4, len(flat))):
                _issue_x(fi, eng8=nc.sync)

            # units B-D weights on gpsimd, per-unit interleaved (fp8
            # first within each unit) so no unit's fp8 weights queue
            # behind a later unit's WAR-blocked bf16 trigger; the waits
            # block only the otherwise-idle gpsimd queue, in order
            ua["w1_sb"] = wbf.tile([P, CB, FQ], bf16, tag="w1", name="w1A")
            nc.gpsimd.dma_start(ua["w1_sb"][:, :, :], ua["w1t"].ap())
            ua["w2_sb"] = wbf.tile([P, FBQ, D_MODEL], bf16, tag="w2", name="w2A")
            nc.gpsimd.dma_start(ua["w2_sb"][:, :, :], ua["w2t"].ap())
            for ui, u in enumerate(units):
                slot = u["slot"]
                if ui == 0:
                    continue
                if u["n8"]:
                    u["w1q8_sb"] = wq8.tile([P, CB, FQ], f8, tag="w1q8", name=f"w1q8{slot}")
                    nc.gpsimd.dma_start(u["w1q8_sb"][:, :, :], u["w1q8"].ap())
                    u["w2q8_sb"] = wq8.tile([P, FBQ, D_MODEL], f8, tag="w2q8", name=f"w2q8{slot}")
                    nc.gpsimd.dma_start(u["w2q8_sb"][:, :, :], u["w2q8"].ap())
                u["w1_sb"] = wbf.tile([P, CB, FQ], bf16, tag="w1", name=f"w1{slot}")
                nc.gpsimd.dma_start(u["w1_sb"][:, :, :], u["w1t"].ap())
                u["w2_sb"] = wbf.tile([P, FBQ, D_MODEL], bf16, tag="w2", name=f"w2{slot}")
                nc.gpsimd.dma_start(u["w2_sb"][:, :, :], u["w2t"].ap())

            # ---- compute: flat tile sequence with lookahead-3 prefetch ----
            for fi, (ui, ti, off, tw, is8) in enumerate(flat):
                u = units[ui]
                ntl = len(u["tiles"])
                if fi + 3 < len(flat):
                    _issue_x(fi + 3)
                last2 = fi >= len(flat) - 2
                ydst = u["yT"].ap()
                xt = pre.pop((ui, ti))
                if is8:
                    ht = h8pool.tile([P, FBQ, TN], f8, tag="h8")
                    for m in range(FBQ):
                        ph = psum.tile([P, TN], f32, tag="ph")
                        for j in range(CB // 2):
                            nc.tensor.matmul(
                                ph[:, :tw],
                                lhsT=u["w1q8_sb"][:, 2 * j:2 * j + 2, ts(m, P)],
                                rhs=xt[:, 2 * j:2 * j + 2, :tw],
                                start=(j == 0), stop=(j == CB // 2 - 1),
                                perf_mode=DR,
                            )
                        nc.scalar.activation(ht[:, m, :tw], ph[:, :tw], Act.Gelu,
                                             bias=u["b1ap"](m),
                                             scale=1.0 / SW)
                    yt = ypool.tile([P, CB, TN], bf16, tag="yt")
                    for c in range(CB):
                        py = psum.tile([P, TN], f32, tag="py")
                        for j in range(FBQ // 2):
                            nc.tensor.matmul(
                                py[:, :tw],
                                lhsT=u["w2q8_sb"][:, 2 * j:2 * j + 2, ts(c, P)],
                                rhs=ht[:, 2 * j:2 * j + 2, :tw],
                                start=(j == 0), stop=(j == FBQ // 2 - 1),
                                perf_mode=DR,
                            )
                        nc.scalar.activation(yt[:, c, :tw], py[:, :tw], Act.Identity,
                                             bias=u["b2ap"](c),
                                             scale=1.0 / SW)
                    if last2:
                        nc.sync.dma_start(ydst[0:64, ds(ti * CB, CB), :], yt[0:64, :, :])
                        nc.scalar.dma_start(ydst[64:96, ds(ti * CB, CB), :], yt[64:96, :, :])
                        nc.gpsimd.dma_start(ydst[96:P, ds(ti * CB, CB), :], yt[96:P, :, :])
                    else:
                        nc.sync.dma_start(ydst[:, ds(ti * CB, CB), :], yt[:, :, :])
                else:
                    ht = hpool.tile([P, FBQ, TN], bf16, tag="ht")
                    for m in range(FBQ):
                        ph = psum.tile([P, TN], f32, tag="ph")
                        for k in range(CB):
                            nc.tensor.matmul(
                                ph[:, :tw],
                                lhsT=u["w1_sb"][:, k, ts(m, P)],
                                rhs=xt[:, k, :tw],
                                start=(k == 0), stop=(k == CB - 1),
                            )
                        nc.scalar.activation(ht[:, m, :tw], ph[:, :tw], Act.Gelu,
                                             bias=u["b1ap"](m))
                    yt = ypool.tile([P, CB, TN], bf16, tag="yt")
                    for c in range(CB):
                        py = psum.tile([P, TN], f32, tag="py")
                        for k in range(FBQ):
                            nc.tensor.matmul(
                                py[:, :tw],
                                lhsT=u["w2_sb"][:, k, ts(c, P)],
                                rhs=ht[:, k, :tw],
                                start=(k == 0), stop=(k == FBQ - 1),
                            )
                        nc.scalar.add(yt[:, c, :tw], py[:, :tw], u["b2ap"](c))
                    if last2:
                        # drain: split the tail stores across the three
                        # rings so the final tiny tile is not serialized
                        # behind a 1MB single-ring store
                        nc.sync.dma_start(ydst[0:64, ds(ti * CB, CB), :], yt[0:64, :, :])
                        nc.scalar.dma_start(ydst[64:96, ds(ti * CB, CB), :], yt[64:96, :, :])
                        nc.gpsimd.dma_start(ydst[96:P, ds(ti * CB, CB), :], yt[96:P, :, :])
                    else:
                        nc.sync.dma_start(ydst[:, ds(ti * CB, CB), :], yt[:, :, :])

    _fix_multiwait_bir(nc)
    _NC_CACHE[cfg] = nc
    return nc


def _route(xf, router_w, k):
    """Replicate the reference router numerics (f32 softmax, top-k, renorm)."""
    logits = xf @ router_w.T.astype(np.float32)          # [T, E]
    m = logits.max(axis=-1, keepdims=True)
    e = np.exp(logits - m, dtype=np.float32)
    probs = e / e.sum(axis=-1, keepdims=True)
    # descending, ties -> lower index first (matches jax.lax.top_k)
    idx = np.argsort(-probs, axis=-1, kind="stable")[:, :k]   # [T, k]
    w = np.take_along_axis(probs, idx, axis=-1)               # [T, k]
    w = w / (w.sum(axis=-1, keepdims=True) + 1e-9)
    return idx, w


def _align16(n):
    return max(P, -(-n // 16) * 16)


def _f8(a):
    return np.clip(a, -240.0, 240.0).astype(ml_dtypes.float8_e4m3)


def _pack_w(w):
    """[C, F] -> partition-contiguous [P, C//P, F] (16KB dma runs)."""
    c, f = w.shape
    return np.ascontiguousarray(w.reshape(c // P, P, f).transpose(1, 0, 2))


def _pack_x(cols, nt):
    """[C, nt*TN] -> tiled [P, nt*CB, TN] (>=4KB dma runs per partition)."""
    c = cols.shape[0]
    a = cols.reshape(CB, P, nt, TN)          # [g, p, t, tok]
    return np.ascontiguousarray(a.transpose(1, 2, 0, 3).reshape(P, nt * CB, TN))


def _unpack_y(arr, nt):
    """[P, nt*CB, TN] -> [C, nt*TN]."""
    a = arr.reshape(P, nt, CB, TN).transpose(2, 0, 1, 3)   # [g, p, t, tok]
    return a.reshape(D_MODEL, nt * TN)


def kernel(x, router_w, expert_w1, expert_b1, expert_w2, expert_b2, top_k):
    x = np.asarray(x)
    router_w = np.asarray(router_w, dtype=np.float32)
    expert_w1 = np.asarray(expert_w1, dtype=np.float32)
    expert_b1 = np.asarray(expert_b1, dtype=np.float32)
    expert_w2 = np.asarray(expert_w2, dtype=np.float32)
    expert_b2 = np.asarray(expert_b2, dtype=np.float32)
    k = int(np.asarray(top_k))
    Bq, Nq, C = x.shape
    Tq = Bq * Nq
    E = expert_w1.shape[0]
    xf = np.ascontiguousarray(x.reshape(Tq, C), dtype=np.float32)

    idx, w = _route(xf, router_w, k)

    # per expert: tokens sorted by combine weight ascending, so the leading
    # (fp8) tiles hold the pairs whose error is most attenuated on combine
    tok_idx, tok_w = [], []
    for e in range(E):
        mask = idx == e
        sel = np.nonzero(mask.any(axis=-1))[0]
        we = (w * mask).sum(axis=-1)[sel].astype(np.float32)
        srt = np.argsort(we, kind="stable")
        tok_idx.append(sel[srt])
        tok_w.append(we[srt])
    counts = np.array([len(s) for s in tok_idx])

    # pair experts by ranked count; slot cap = pair max
    order = np.argsort(-counts, kind="stable")
    pairs = [(int(order[2 * s]), int(order[2 * s + 1])) for s in range(4)]
    pcaps = [_align16(int(counts[p[0]])) for p in pairs]
    # fp8 column budget: half-tiles round-robin, largest pairs first
    f8_cols = int(os.environ.get("BASS_MOE_F8", F8_COLS))
    pn8 = [0, 0, 0, 0]
    for i in range(f8_cols // HT):
        pn8[(i // 2) % 4] += HT
    pn8 = [min(n, pcaps[i] // HT * HT) for i, n in enumerate(pn8)]
    # compute order: largest tail width first, so the drain tile is small
    tails = [((c - 1) % TN) + 1 for c in pcaps]
    corder = sorted(range(4), key=lambda i: -tails[i])
    cfg = tuple((pcaps[i], pn8[i]) for i in corder)

    nc = _build_moe_kernel(cfg)

    # one xT8/xT pair per expert, shared by its four quarter-units
    x8s, xbs, slot_of = {}, {}, {}
    for si, pi in enumerate(corder):
        cap, n8 = pcaps[pi], pn8[pi]
        tiles = _tiles_of(cap, n8)
        nt8 = sum(1 for t in tiles if t[2])
        ntb = len(tiles) - nt8
        for e in pairs[pi]:
            slot_of[e] = si
            cols = xf[tok_idx[e]].T          # [C, count]
            cnt = cols.shape[1]
            c8 = np.zeros((C, nt8 * TN), dtype=np.float32)
            c8[:, :min(cnt, n8)] = cols[:, :n8]
            cb = np.zeros((C, ntb * TN), dtype=np.float32)
            if cnt > n8:
                cb[:, :cnt - n8] = cols[:, n8:]
            x8s[e] = _pack_x(c8, nt8).astype(ml_dtypes.float8_e4m3) if nt8 else None
            xbs[e] = _pack_x(cb, ntb).astype(np.float16) if ntb else None

    in_maps = [dict() for _ in range(N_CORES)]
    biases = [np.zeros((P, 4 * 2 * FBQ), dtype=np.float32) for _ in range(N_CORES)]
    placement = {}          # (expert, quarter) -> (core, slot name)
    for si, pi in enumerate(corder):
        slot = SLOTS[si]
        n8 = pn8[pi]
        for core in range(N_CORES):
            e = pairs[pi][0] if core < 4 else pairs[pi][1]
            q = core % 4
            placement[(e, q)] = (core, slot)
            f0, f1 = q * FQ, (q + 1) * FQ
            b2 = expert_b2[e] if q == 0 else np.zeros(C, dtype=np.float32)
            biases[core][:, si * 2 * FBQ: si * 2 * FBQ + FBQ] = \
                expert_b1[e, f0:f1].reshape(FBQ, P).T
            biases[core][:, si * 2 * FBQ + FBQ: (si + 1) * 2 * FBQ] = \
                b2.reshape(CB, P).T
            w1s = np.ascontiguousarray(expert_w1[e, f0:f1].T)   # [C, FQ]
            w2s = np.ascontiguousarray(expert_w2[e, :, f0:f1].T)  # [FQ, C]
            m = {
                f"w1t{slot}": _pack_w(w1s).astype(np.float16),
                f"w2t{slot}": _pack_w(w2s).astype(np.float16),
            }
            if xbs[e] is not None:
                m[f"xT{slot}"] = xbs[e]
            if n8:
                m[f"xT8{slot}"] = x8s[e]
                m[f"w1q8{slot}"] = _f8(_pack_w(w1s) * SW)
                m[f"w2q8{slot}"] = _f8(_pack_w(w2s) * SW)
            in_maps[core].update(m)
    for core in range(N_CORES):
        in_maps[core]["biases"] = biases[core]

    trace = os.environ.get("BASS_MOE_TRACE") == "1"
    res = run_bass_kernel_spmd(
        nc, in_maps, core_ids=list(range(N_CORES)),
        trace=trace,
        tmpdir=os.environ.get("BASS_MOE_TMPDIR") if trace else None,
    )
    if trace:
        kernel.last_exec_time_ns = res.exec_time_ns
        kernel.last_trace = (res.instructions_and_trace or (None, None))[1]

    out = np.zeros((Tq, C), dtype=np.float32)
    for e in range(E):
        cnt = counts[e]
        if not cnt:
            continue
        si = slot_of[e]
        cap, n8 = cfg[si][0], cfg[si][1]
        nt = len(_tiles_of(cap, n8))
        acc = np.zeros((cnt, C), dtype=np.float32)
        for q in range(4):
            core, slot = placement[(e, q)]
            yr = _unpack_y(res.results[core][f"yT{slot}"].astype(np.float32), nt)
            nt8 = -(-n8 // TN)
            ycols = np.concatenate([yr[:, :n8], yr[:, nt8 * TN:nt8 * TN + (cap - n8)]], axis=1)
            acc += ycols[:, :cnt].T
        out[tok_idx[e]] += acc * tok_w[e][:, None]
    return out.reshape(Bq, Nq, C).astype(x.dtype)


# revision 19
# speedup vs baseline: 1.0888x; 1.0888x over previous
"""MoE layer (top-k routing) on 8 Trainium2 NeuronCores.

Expert-parallel per the sharding hint: the host computes router softmax +
top-k (0.1% of FLOPs) and realizes the "all-to-all dispatch by expert
assignment" while building the per-core SPMD input maps; each core runs
expert FFN work (fp32 PSUM accumulation); the host applies the combine
weights and scatter-adds results back to [B,N,C].

Load balance: each expert's FFN is split along D_FF into four quarter-units
(exact: gelu is elementwise over F and GEMM2 contracts F, so the four
partial y's just add). The 32 quarter-units are assigned four per core, one
per slot class A-D: each slot holds one expert pair's quarters (cores 0-3
take the larger expert of the pair, 4-7 the smaller), padded to the pair
max, within ~1% of the perfect-balance floor.

Mixed precision: each expert's tokens are sorted by combine weight
ascending; the first n8 columns of each slot run both GEMMs as naked
fp8-e4m3 DoubleRow matmuls (virtual K=256, measured ~1.9x bf16 per tile),
the rest in fp16 (fp16 runs at bf16 speed on the PE but is exact in its
e10m11 internal format, freeing error budget for more fp8 tiles). A
pair's output error is scaled by its combine weight, so putting
low-weight pairs on the fp8 path keeps the end-to-end rel err ~1.73e-2
(budget 2e-2) while cutting PE busy time ~13%.

All dram tensors use partition-contiguous tiled layouts ([P, tiles*CB, TN]
for x/y, [P, CB, FQ]-style for weights) so every DMA moves 4-16KB runs per
partition; the f-major rearranged layouts previously produced 0.25-1KB
packets that halved ring throughput and starved the PE at startup.

DMA rings: sync carries the first unit's fp8 weights (it is the earliest
queue to start, ~9us) then x tiles in consumption order and y stores;
scalar carries biases + the first unit's bf16 weights (no WAR waits ahead
of the activations); gpsimd carries units B-D weights, WAR-paced by the
double-buffered weight pools, which blocks nothing since gpsimd has no
other work. Units are computed largest-tail first so the final tile's
store drain is minimal.
"""

import json
import os
import sys
import types

import numpy as np
import ml_dtypes

D_MODEL = 1024
D_FF = 4096
N_EXPERTS = 8
N_CORES = 8

P = 128
CB = D_MODEL // P      # 8 c-blocks of 128
FQ = D_FF // 4         # F quarter = 1024
FBQ = FQ // P          # 8 f-blocks per quarter
TN = 512               # token tile (matmul moving free dim / one PSUM bank)
HT = 256               # half-tile granularity for the fp8 column budget
SLOTS = ("A", "B", "C", "D")
SW = 4096.0            # fp8 weight pre-scale (max |w|*SW ~ 140 < 240)
F8_COLS = 4096         # total fp8 token columns across the 4 slots (8 tiles)


def _shim_axon_hooks():
    """Register the NTFF profile hook bass_utils looks for under axon; the
    image's `antenv` stub lacks `axon_hooks`."""
    if "antenv.axon_hooks" in sys.modules:
        return
    try:
        import trn_agent_boot.trn_boot as _tb
        hook = _tb._ntff_profile_via_ctypes("/opt/axon/libaxon_pjrt.so")
    except Exception:
        hook = None
    mod = types.ModuleType("antenv.axon_hooks")
    mod.get_axon_ntff_profile_hook = lambda: hook
    mod.set_axon_ntff_profile_hook = lambda h: None
    sys.modules["antenv.axon_hooks"] = mod


_shim_axon_hooks()

import concourse.bass as bass            # noqa: E402
import concourse.tile as tile            # noqa: E402
from concourse import mybir              # noqa: E402
from concourse.bass import ds, ts        # noqa: E402
from concourse.bass_utils import run_bass_kernel_spmd  # noqa: E402


def _fix_multiwait_bir(nc):
    """Split instructions carrying >1 sync wait (the TileContext tail drain)
    into single-wait NoOps; this walrus build rejects multi-wait CTRL
    instructions."""
    raw = bass.Bass.to_json_bytes(nc)
    d = json.loads(raw)
    for f in d["functions"]:
        for b in f["blocks"]:
            out = []
            for i in b["instructions"]:
                si = i.get("sync_info") or {}
                waits = si.get("on_wait") or []
                if len(waits) > 1:
                    for k, w in enumerate(waits[:-1]):
                        out.append({
                            "name": f"{i['name']}_wsplit{k}",
                            "engine": i["engine"],
                            "ins": [], "outs": [],
                            "opcode": "NoOp",
                            "sync_info": {"on_update": [], "on_wait": [w]},
                        })
                    si["on_wait"] = [waits[-1]]
                out.append(i)
            b["instructions"] = out
    fixed = json.dumps(d).encode()
    nc.to_json_bytes = lambda: fixed


_NC_CACHE = {}


def _tiles_of(cap, n8):
    """(off, tw, is_fp8) tiles: fp8 columns first (512s then a possible 256
    half-tile), then bf16 512-tiles with a ragged tail."""
    tiles = []
    off = 0
    while off < n8:
        tw = min(TN, n8 - off)
        tiles.append((off, tw, True))
        off += tw
    while off < cap:
        tw = min(TN, cap - off)
        tiles.append((off, tw, False))
        off += tw
    return tiles


def _build_moe_kernel(cfg):
    """cfg: tuple of (cap, n8) per slot in compute order. SPMD x8."""
    if cfg in _NC_CACHE:
        return _NC_CACHE[cfg]

    bf16 = mybir.dt.float16
    f8 = mybir.dt.float8e4
    f32 = mybir.dt.float32
    Act = mybir.ActivationFunctionType
    DR = mybir.MatmulPerfMode.DoubleRow

    nc = bass.Bass("TRN2", target_bir_lowering=False, debug=False,
                   num_devices=N_CORES)

    units = []
    for slot, (cap, n8) in zip(SLOTS, cfg):
        u = {"cap": cap, "n8": n8, "slot": slot}
        u["tiles"] = _tiles_of(cap, n8)
        nt8 = sum(1 for t in u["tiles"] if t[2])
        ntb = len(u["tiles"]) - nt8
        u["nt8"], u["ntb"] = nt8, ntb
        if n8:
            u["xT8"] = nc.declare_dram_parameter(f"xT8{slot}", [P, nt8 * CB, TN], f8, isOutput=False)
            u["w1q8"] = nc.declare_dram_parameter(f"w1q8{slot}", [P, CB, FQ], f8, isOutput=False)
            u["w2q8"] = nc.declare_dram_parameter(f"w2q8{slot}", [P, FBQ, D_MODEL], f8, isOutput=False)
        if ntb:
            u["xT"] = nc.declare_dram_parameter(f"xT{slot}", [P, ntb * CB, TN], bf16, isOutput=False)
        u["w1t"] = nc.declare_dram_parameter(f"w1t{slot}", [P, CB, FQ], bf16, isOutput=False)
        u["w2t"] = nc.declare_dram_parameter(f"w2t{slot}", [P, FBQ, D_MODEL], bf16, isOutput=False)
        # partials return as bf16: halves the output DMA so total traffic
        # stays under the chip's P0 power-throttle trigger; host sums in f32
        u["yT"] = nc.declare_dram_parameter(f"yT{slot}", [P, len(u["tiles"]) * CB, TN], bf16, isOutput=True)
        units.append(u)
    # all biases in one partition-contiguous tensor: the old per-slot
    # "(g p) -> p g" rearrange emitted 8192 four-byte DMA packets that
    # clogged a ring for tens of us
    bias_d = nc.declare_dram_parameter("biases", [P, 4 * 2 * FBQ], f32, isOutput=False)

    ua = units[0]
    with tile.TileContext(nc) as tc:
        with (
            tc.tile_pool(name="wbf", bufs=2) as wbf,
            tc.tile_pool(name="wq8", bufs=2) as wq8,
            tc.tile_pool(name="bias", bufs=1) as bpool,
            tc.tile_pool(name="xin", bufs=4) as xpool,
            tc.tile_pool(name="x8in", bufs=4) as x8pool,
            tc.tile_pool(name="hbuf", bufs=2) as hpool,
            tc.tile_pool(name="h8buf", bufs=2) as h8pool,
            tc.tile_pool(name="yout", bufs=2) as ypool,
            tc.tile_pool(name="psum", bufs=4, space="PSUM") as psum,
        ):
            # ---- weight/bias loads.
            # unit A's startup-critical fp8 tensors are split by partition
            # halves across the sync (starts ~9us) and scalar (~11us) rings:
            # the per-queue packet rate caps one ring at ~130-190GB/s, so
            # two queues halve the time to the first matmul. biases go
            # first on gpsimd (one 32KB dma). all bf16/deferred weights go
            # on gpsimd, WAR-paced by the double-buffered pools, which
            # blocks nothing since gpsimd has no other work.
            bt = bpool.tile([P, 4 * 2 * FBQ], f32, tag="bias", name="bias")
            nc.gpsimd.dma_start(bt[:, :], bias_d.ap())
            for ui, u in enumerate(units):
                u["b1ap"] = lambda m, _s=ui * 2 * FBQ: bt[:, ds(_s + m, 1)]
                u["b2ap"] = lambda c, _s=ui * 2 * FBQ + FBQ: bt[:, ds(_s + c, 1)]
            u = None
            # flat tile list in natural order (each unit: fp8 tiles then
            # bf16); x loads are issued 3 tiles ahead of consumption so no
            # x trigger queues behind a y-store trigger whose data is not
            # ready yet (the in-order sync engine would collapse the
            # prefetch pipeline at unit boundaries and at the drain)
            flat = []
            for ui_, u_ in enumerate(units):
                for ti_, tt in enumerate(u_["tiles"]):
                    flat.append((ui_, ti_) + tt)
            pre = {}
            # startup-critical unit-A tensors split by partition halves
            # across the sync (starts ~9us) and scalar (~11us) rings: the
            # per-queue packet rate caps one ring at ~130-190GB/s
            H = P // 2
            if ua["n8"]:
                ua["w1q8_sb"] = wq8.tile([P, CB, FQ], f8, tag="w1q8", name="w1q8A")
                ua["w2q8_sb"] = wq8.tile([P, FBQ, D_MODEL], f8, tag="w2q8", name="w2q8A")
                x0 = x8pool.tile([P, CB, TN], f8, tag="x8", name="x0")
                nc.sync.dma_start(ua["w1q8_sb"][0:H, :, :], ua["w1q8"].ap()[0:H, :, :])
                nc.scalar.dma_start(ua["w1q8_sb"][H:P, :, :], ua["w1q8"].ap()[H:P, :, :])
                nc.sync.dma_start(x0[0:H, :, :], ua["xT8"].ap()[0:H, ds(0, CB), :])
                nc.scalar.dma_start(x0[H:P, :, :], ua["xT8"].ap()[H:P, ds(0, CB), :])
                nc.sync.dma_start(ua["w2q8_sb"][0:H, :, :], ua["w2q8"].ap()[0:H, :, :])
                nc.scalar.dma_start(ua["w2q8_sb"][H:P, :, :], ua["w2q8"].ap()[H:P, :, :])
                pre[(0, 0)] = x0

            def _issue_x(fi, eng8=None):
                ui_, ti_, off_, tw_, is8_ = flat[fi]
                if (ui_, ti_) in pre:
                    return
                u_ = units[ui_]
                if is8_:
                    xt_ = x8pool.tile([P, CB, TN], f8, tag="x8")
                    (eng8 or nc.scalar).dma_start(xt_[:, :, :], u_["xT8"].ap()[:, ds(ti_ * CB, CB), :])
                else:
                    tb_ = ti_ - u_["nt8"]
                    xt_ = xpool.tile([P, CB, TN], bf16, tag="xt")
                    nc.sync.dma_start(xt_[:, :, :], u_["xT"].ap()[:, ds(tb_ * CB, CB), :])
                pre[(ui_, ti_)] = xt_

            for fi in range(min(4, len(flat))):
                _issue_x(fi, eng8=nc.sync)

            # units B-D weights on gpsimd, per-unit interleaved (fp8
            # first within each unit) so no unit's fp8 weights queue
            # behind a later unit's WAR-blocked bf16 trigger; the waits
            # block only the otherwise-idle gpsimd queue, in order
            ua["w1_sb"] = wbf.tile([P, CB, FQ], bf16, tag="w1", name="w1A")
            nc.gpsimd.dma_start(ua["w1_sb"][:, :, :], ua["w1t"].ap())
            ua["w2_sb"] = wbf.tile([P, FBQ, D_MODEL], bf16, tag="w2", name="w2A")
            nc.gpsimd.dma_start(ua["w2_sb"][:, :, :], ua["w2t"].ap())
            for ui, u in enumerate(units):
                slot = u["slot"]
                if ui == 0:
                    continue
                if u["n8"]:
                    u["w1q8_sb"] = wq8.tile([P, CB, FQ], f8, tag="w1q8", name=f"w1q8{slot}")
                    nc.gpsimd.dma_start(u["w1q8_sb"][:, :, :], u["w1q8"].ap())
                    u["w2q8_sb"] = wq8.tile([P, FBQ, D_MODEL], f8, tag="w2q8", name=f"w2q8{slot}")
                    nc.gpsimd.dma_start(u["w2q8_sb"][:, :, :], u["w2q8"].ap())
                u["w1_sb"] = wbf.tile([P, CB, FQ], bf16, tag="w1", name=f"w1{slot}")
                nc.gpsimd.dma_start(u["w1_sb"][:, :, :], u["w1t"].ap())
                u["w2_sb"] = wbf.tile([P, FBQ, D_MODEL], bf16, tag="w2", name=f"w2{slot}")
                nc.gpsimd.dma_start(u["w2_sb"][:, :, :], u["w2t"].ap())

            # ---- compute: flat tile sequence with lookahead-3 prefetch ----
            for fi, (ui, ti, off, tw, is8) in enumerate(flat):
                u = units[ui]
                ntl = len(u["tiles"])
                if fi + 3 < len(flat):
                    _issue_x(fi + 3)
                last2 = fi >= len(flat) - 2
                ydst = u["yT"].ap()
                xt = pre.pop((ui, ti))
                if is8:
                    ht = h8pool.tile([P, FBQ, TN], f8, tag="h8")
                    for m in range(FBQ):
                        ph = psum.tile([P, TN], f32, tag="ph")
                        for j in range(CB // 2):
                            nc.tensor.matmul(
                                ph[:, :tw],
                                lhsT=u["w1q8_sb"][:, 2 * j:2 * j + 2, ts(m, P)],
                                rhs=xt[:, 2 * j:2 * j + 2, :tw],
                                start=(j == 0), stop=(j == CB // 2 - 1),
                                perf_mode=DR,
                            )
                        nc.scalar.activation(ht[:, m, :tw], ph[:, :tw], Act.Gelu,
                                             bias=u["b1ap"](m),
                                             scale=1.0 / SW)
                    yt = ypool.tile([P, CB, TN], bf16, tag="yt")
                    for c in range(CB):
                        py = psum.tile([P, TN], f32, tag="py")
                        for j in range(FBQ // 2):
                            nc.tensor.matmul(
                                py[:, :tw],
                                lhsT=u["w2q8_sb"][:, 2 * j:2 * j + 2, ts(c, P)],
                                rhs=ht[:, 2 * j:2 * j + 2, :tw],
                                start=(j == 0), stop=(j == FBQ // 2 - 1),
                                perf_mode=DR,
                            )
                        nc.scalar.activation(yt[:, c, :tw], py[:, :tw], Act.Identity,
                                             bias=u["b2ap"](c),
                                             scale=1.0 / SW)
                    if last2:
                        nc.sync.dma_start(ydst[0:64, ds(ti * CB, CB), :], yt[0:64, :, :])
                        nc.scalar.dma_start(ydst[64:96, ds(ti * CB, CB), :], yt[64:96, :, :])
                        nc.gpsimd.dma_start(ydst[96:P, ds(ti * CB, CB), :], yt[96:P, :, :])
                    else:
                        nc.sync.dma_start(ydst[:, ds(ti * CB, CB), :], yt[:, :, :])
                else:
                    ht = hpool.tile([P, FBQ, TN], bf16, tag="ht")
                    for m in range(FBQ):
                        ph = psum.tile([P, TN], f32, tag="ph")
                        for k in range(CB):
                            nc.tensor.matmul(
                                ph[:, :tw],
                                lhsT=u["w1_sb"][:, k, ts(m, P)],
                                rhs=xt[:, k, :tw],
                                start=(k == 0), stop=(k == CB - 1),
                            )
                        nc.scalar.activation(ht[:, m, :tw], ph[:, :tw], Act.Gelu,
                                             bias=u["b1ap"](m))
                    yt = ypool.tile([P, CB, TN], bf16, tag="yt")
                    for c in range(CB):
                        py = psum.tile([P, TN], f32, tag="py")
                        for k in range(FBQ):
                            nc.tensor.matmul(
                                py[:, :tw],
                                lhsT=u["w2_sb"][:, k, ts(c, P)],
                                rhs=ht[:, k, :tw],
                                start=(k == 0), stop=(k == FBQ - 1),
                            )
                        nc.scalar.add(yt[:, c, :tw], py[:, :tw], u["b2ap"](c))
                    if last2:
                        # drain: split the tail stores across the three
                        # rings so the final tiny tile is not serialized
                        # behind a 1MB single-ring store
                        nc.sync.dma_start(ydst[0:64, ds(ti * CB, CB), :], yt[0:64, :, :])
                        nc.scalar.dma_start(ydst[64:96, ds(ti * CB, CB), :], yt[64:96, :, :])
                        nc.gpsimd.dma_start(ydst[96:P, ds(ti * CB, CB), :], yt[96:P, :, :])
                    else:
                        nc.sync.dma_start(ydst[:, ds(ti * CB, CB), :], yt[:, :, :])

    _fix_multiwait_bir(nc)
    _NC_CACHE[cfg] = nc
    return nc


def _route(xf, router_w, k):
    """Replicate the reference router numerics (f32 softmax, top-k, renorm)."""
    logits = xf @ router_w.T.astype(np.float32)          # [T, E]
    m = logits.max(axis=-1, keepdims=True)
    e = np.exp(logits - m, dtype=np.float32)
    probs = e / e.sum(axis=-1, keepdims=True)
    # descending, ties -> lower index first (matches jax.lax.top_k)
    idx = np.argsort(-probs, axis=-1, kind="stable")[:, :k]   # [T, k]
    w = np.take_along_axis(probs, idx, axis=-1)               # [T, k]
    w = w / (w.sum(axis=-1, keepdims=True) + 1e-9)
    return idx, w


def _align16(n):
    return max(P, -(-n // 16) * 16)


def _f8(a):
    return np.clip(a, -240.0, 240.0).astype(ml_dtypes.float8_e4m3)


def _pack_w(w):
    """[C, F] -> partition-contiguous [P, C//P, F] (16KB dma runs)."""
    c, f = w.shape
    return np.ascontiguousarray(w.reshape(c // P, P, f).transpose(1, 0, 2))


def _pack_x(cols, nt):
    """[C, nt*TN] -> tiled [P, nt*CB, TN] (>=4KB dma runs per partition)."""
    c = cols.shape[0]
    a = cols.reshape(CB, P, nt, TN)          # [g, p, t, tok]
    return np.ascontiguousarray(a.transpose(1, 2, 0, 3).reshape(P, nt * CB, TN))


def _unpack_y(arr, nt):
    """[P, nt*CB, TN] -> [C, nt*TN]."""
    a = arr.reshape(P, nt, CB, TN).transpose(2, 0, 1, 3)   # [g, p, t, tok]
    return a.reshape(D_MODEL, nt * TN)


def kernel(x, router_w, expert_w1, expert_b1, expert_w2, expert_b2, top_k):
    x = np.asarray(x)
    router_w = np.asarray(router_w, dtype=np.float32)
    expert_w1 = np.asarray(expert_w1, dtype=np.float32)
    expert_b1 = np.asarray(expert_b1, dtype=np.float32)
    expert_w2 = np.asarray(expert_w2, dtype=np.float32)
    expert_b2 = np.asarray(expert_b2, dtype=np.float32)
    k = int(np.asarray(top_k))
    Bq, Nq, C = x.shape
    Tq = Bq * Nq
    E = expert_w1.shape[0]
    xf = np.ascontiguousarray(x.reshape(Tq, C), dtype=np.float32)

    idx, w = _route(xf, router_w, k)

    # per expert: tokens sorted by combine weight ascending, so the leading
    # (fp8) tiles hold the pairs whose error is most attenuated on combine
    tok_idx, tok_w = [], []
    for e in range(E):
        mask = idx == e
        sel = np.nonzero(mask.any(axis=-1))[0]
        we = (w * mask).sum(axis=-1)[sel].astype(np.float32)
        srt = np.argsort(we, kind="stable")
        tok_idx.append(sel[srt])
        tok_w.append(we[srt])
    counts = np.array([len(s) for s in tok_idx])

    # pair experts by ranked count; slot cap = pair max
    order = np.argsort(-counts, kind="stable")
    pairs = [(int(order[2 * s]), int(order[2 * s + 1])) for s in range(4)]
    pcaps = [_align16(int(counts[p[0]])) for p in pairs]
    # fp8 column budget: half-tiles round-robin, largest pairs first
    f8_cols = int(os.environ.get("BASS_MOE_F8", F8_COLS))
    pn8 = [0, 0, 0, 0]
    for i in range(f8_cols // HT):
        pn8[(i // 2) % 4] += HT
    pn8 = [min(n, pcaps[i] // HT * HT) for i, n in enumerate(pn8)]
    # compute order: largest tail width first, so the drain tile is small
    tails = [((c - 1) % TN) + 1 for c in pcaps]
    corder = sorted(range(4), key=lambda i: -tails[i])
    cfg = tuple((pcaps[i], pn8[i]) for i in corder)

    nc = _build_moe_kernel(cfg)

    # one xT8/xT pair per expert, shared by its four quarter-units
    x8s, xbs, slot_of = {}, {}, {}
    for si, pi in enumerate(corder):
        cap, n8 = pcaps[pi], pn8[pi]
        tiles = _tiles_of(cap, n8)
        nt8 = sum(1 for t in tiles if t[2])
        ntb = len(tiles) - nt8
        for e in pairs[pi]:
            slot_of[e] = si
            cols = xf[tok_idx[e]].T          # [C, count]
            cnt = cols.shape[1]
            c8 = np.zeros((C, nt8 * TN), dtype=np.float32)
            c8[:, :min(cnt, n8)] = cols[:, :n8]
            cb = np.zeros((C, ntb * TN), dtype=np.float32)
            if cnt > n8:
                cb[:, :cnt - n8] = cols[:, n8:]
            x8s[e] = _pack_x(c8, nt8).astype(ml_dtypes.float8_e4m3) if nt8 else None
            xbs[e] = _pack_x(cb, ntb).astype(np.float16) if ntb else None

    in_maps = [dict() for _ in range(N_CORES)]
    biases = [np.zeros((P, 4 * 2 * FBQ), dtype=np.float32) for _ in range(N_CORES)]
    placement = {}          # (expert, quarter) -> (core, slot name)
    for si, pi in enumerate(corder):
        slot = SLOTS[si]
        n8 = pn8[pi]
        for core in range(N_CORES):
            e = pairs[pi][0] if core < 4 else pairs[pi][1]
            q = core % 4
            placement[(e, q)] = (core, slot)
            f0, f1 = q * FQ, (q + 1) * FQ
            b2 = expert_b2[e] if q == 0 else np.zeros(C, dtype=np.float32)
            biases[core][:, si * 2 * FBQ: si * 2 * FBQ + FBQ] = \
                expert_b1[e, f0:f1].reshape(FBQ, P).T
            biases[core][:, si * 2 * FBQ + FBQ: (si + 1) * 2 * FBQ] = \
                b2.reshape(CB, P).T
            w1s = np.ascontiguousarray(expert_w1[e, f0:f1].T)   # [C, FQ]
            w2s = np.ascontiguousarray(expert_w2[e, :, f0:f1].T)  # [FQ, C]
            m = {
                f"w1t{slot}": _pack_w(w1s).astype(np.float16),
                f"w2t{slot}": _pack_w(w2s).astype(np.float16),
            }
            if xbs[e] is not None:
                m[f"xT{slot}"] = xbs[e]
            if n8:
                m[f"xT8{slot}"] = x8s[e]
                m[f"w1q8{slot}"] = _f8(_pack_w(w1s) * SW)
                m[f"w2q8{slot}"] = _f8(_pack_w(w2s) * SW)
            in_maps[core].update(m)
    for core in range(N_CORES):
        in_maps[core]["biases"] = biases[core]

    trace = os.environ.get("BASS_MOE_TRACE") == "1"
    res = run_bass_kernel_spmd(
        nc, in_maps, core_ids=list(range(N_CORES)),
        trace=trace,
        tmpdir=os.environ.get("BASS_MOE_TMPDIR") if trace else None,
    )
    if trace:
        kernel.last_exec_time_ns = res.exec_time_ns
        kernel.last_trace = (res.instructions_and_trace or (None, None))[1]

    out = np.zeros((Tq, C), dtype=np.float32)
    for e in range(E):
        cnt = counts[e]
        if not cnt:
            continue
        si = slot_of[e]
        cap, n8 = cfg[si][0], cfg[si][1]
        nt = len(_tiles_of(cap, n8))
        acc = np.zeros((cnt, C), dtype=np.float32)
        for q in range(4):
            core, slot = placement[(e, q)]
            yr = _unpack_y(res.results[core][f"yT{slot}"].astype(np.float32), nt)
            nt8 = -(-n8 // TN)
            ycols = np.concatenate([yr[:, :n8], yr[:, nt8 * TN:nt8 * TN + (cap - n8)]], axis=1)
            acc += ycols[:, :cnt].T
        out[tok_idx[e]] += acc * tok_w[e][:, None]
    return out.reshape(Bq, Nq, C).astype(x.dtype)
